# revision 9
# baseline (speedup 1.0000x reference)
"""Trainium2 Bass kernel for 2-layer GATv2 (nn_GAT_25958782337775).

Strategy (8 NeuronCores, SPMD):
  - Nodes are sorted by (deg_lo, deg_hi) of their incoming edges, grouped into
    128-node groups of near-uniform in-degree, and dealt round-robin to the 8
    cores (destination-node sharding per the hint).
  - Per group, incoming edges live in dense [128 nodes, D slots] tensors:
    source features are fetched with dma_gather (bf16/f32 rows), the
    segment-softmax + weighted sum become plain free-dim reductions, and
    invalid slots are masked after exp.  No scatter is ever needed.
  - 3 device launches: (A) per-core feature transforms x->XL,XR;
    (B) layer-1 edge aggregation -> h; (C) h->HL,HR transforms + layer-2
    edge aggregation + log_softmax.  The host only reshards/casts between
    launches (index bookkeeping; all FLOPs happen on device).

Numerics: layer-1 tables bf16 (values O(1)); layer-2 tables f32 (64-col rows
are 256B in f32, so full precision costs no extra DMA).  Softmax is computed
without the max-subtraction (logits are O(1); exactly equal in exact
arithmetic, matches reference to fp rounding).
"""

import sys

sys.path.insert(0, "/opt/trn_rl_repo")

from contextlib import ExitStack

import numpy as np
import ml_dtypes

import concourse.bacc as bacc
import concourse.tile as tile
from concourse import mybir, library_config
from concourse.bass_utils import run_bass_kernel_spmd

BF16 = ml_dtypes.bfloat16
NCORES = 8
SPLIT = 32768
P = 128
EPS = 1e-16
NEG_SLOPE = 0.2

_TRACE = {"on": False, "exec_ns": 0.0}  # test.py can flip _TRACE["on"]


# --------------------------------------------------------------------------
# host-side graph preprocessing
# --------------------------------------------------------------------------

def _prep_graph(edge_index: np.ndarray, n: int):
    """Group nodes by in-degree, build per-core dense gather schedules."""
    src = np.concatenate([edge_index[0], np.arange(n, dtype=np.int64)])
    dst = np.concatenate([edge_index[1], np.arange(n, dtype=np.int64)])

    # edges sorted by (dst, src >= SPLIT): per-dst runs with low srcs first
    comb = np.lexsort(((src >= SPLIT).astype(np.int64), dst))
    src_s = src[comb]
    dst_s = dst[comb]

    deg = np.bincount(dst_s, minlength=n)
    deg_lo = np.bincount(dst_s[src_s < SPLIT], minlength=n)
    deg_hi = deg - deg_lo
    starts = np.zeros(n + 1, dtype=np.int64)
    np.cumsum(deg, out=starts[1:])

    # node order: pad nodes (deg 0) would come first; all real nodes have >=1
    node_order = np.lexsort((deg_hi, deg_lo))  # ascending
    ng = -(-n // P) // NCORES * NCORES  # groups, multiple of NCORES
    while ng * P < n:
        ng += NCORES
    n_pad = ng * P
    # pad node ids = -1, placed FIRST (deg 0 sorts first conceptually)
    padded = np.concatenate([np.full(n_pad - n, -1, dtype=np.int64), node_order])

    g_per_core = ng // NCORES
    # group j -> core j % 8, slot j // 8
    node_list = np.zeros((NCORES, g_per_core * P), dtype=np.int64)
    for j in range(ng):
        c, s = j % NCORES, j // NCORES
        node_list[c, s * P:(s + 1) * P] = padded[j * P:(j + 1) * P]

    # per-slot schedule: D_lo[s], D_hi[s] = max over cores
    d_lo = np.zeros(g_per_core, dtype=np.int64)
    d_hi = np.zeros(g_per_core, dtype=np.int64)
    for s in range(g_per_core):
        ids = node_list[:, s * P:(s + 1) * P].reshape(-1)
        real = ids[ids >= 0]
        if len(real):
            d_lo[s] = deg_lo[real].max()
            d_hi[s] = deg_hi[real].max()
    d_tot = d_lo + d_hi

    # build per-core wrapped idx arrays + masks
    idx_lo_cores, idx_hi_cores, mask_cores = [], [], []
    for c in range(NCORES):
        lo_blocks, hi_blocks, masks = [], [], []
        for s in range(g_per_core):
            ids = node_list[c, s * P:(s + 1) * P]
            dlo, dhi = int(d_lo[s]), int(d_hi[s])
            lo_idx = np.zeros((dlo, P), dtype=np.int64)   # k-major [k, p]
            hi_idx = np.zeros((dhi, P), dtype=np.int64)
            m = np.zeros((P, dlo + dhi), dtype=np.float32)
            for p in range(P):
                nid = ids[p]
                if nid < 0:
                    continue
                a, b = starts[nid], starts[nid + 1]
                es = src_s[a:b]
                nlo = int(deg_lo[nid])
                lo_idx[:nlo, p] = es[:nlo]
                hi_idx[: int(deg_hi[nid]), p] = es[nlo:] - SPLIT
                m[p, :nlo] = 1.0
                m[p, dlo:dlo + int(deg_hi[nid])] = 1.0
            lo_blocks.append(lo_idx.reshape(-1))
            hi_blocks.append(hi_idx.reshape(-1))
            masks.append(m)
        idx_lo_cores.append(lo_blocks)
        idx_hi_cores.append(hi_blocks)
        mask_cores.append(masks)

    return dict(
        node_list=node_list, d_lo=d_lo, d_hi=d_hi, d_tot=d_tot,
        g_per_core=g_per_core, n_pad_nodes=g_per_core * P,
        idx_lo=idx_lo_cores, idx_hi=idx_hi_cores, masks=mask_cores,
    )


def _wrap_idx(flat: np.ndarray) -> np.ndarray:
    """[m] -> [128, m//16] int16 layout: element m at [m%16, m//16], tiled x8."""
    assert len(flat) % 16 == 0
    arr = flat.astype(np.int16).reshape(-1, 16).T  # [16, m//16]
    return np.tile(arr, (8, 1))


def _concat_idx(blocks):
    cols = [_wrap_idx(b) if len(b) else np.zeros((128, 0), np.int16) for b in blocks]
    return np.ascontiguousarray(np.concatenate(cols, axis=1)) if cols else np.zeros((128, 0), np.int16)


# --------------------------------------------------------------------------
# launch builders
# --------------------------------------------------------------------------

def _run(nc, in_maps):
    if _TRACE["on"]:
        # No NTFF capture in this container; use the instruction cost model
        # (TimelineSim) for the per-launch device-time estimate.
        from concourse.timeline_sim import TimelineSim
        tl = TimelineSim(nc, trace=False)
        ns = tl.simulate()
        _TRACE["exec_ns"] += ns
        _TRACE.setdefault("per_launch", []).append(ns)
    res = run_bass_kernel_spmd(
        nc, in_maps, core_ids=list(range(NCORES)), trace=False,
    )
    return res.results


def _build_transform1(n_nodes, din, fout):
    """XL = x @ Wl, XR = x @ Wr for this core's nodes. xT [din, n_nodes] bf16."""
    nc = bacc.Bacc("TRN2", target_bir_lowering=False, debug=False, num_devices=NCORES)
    xT = nc.dram_tensor("xT", [din, n_nodes], mybir.dt.bfloat16, kind="ExternalInput").ap()
    wl = nc.dram_tensor("wl", [din, fout], mybir.dt.bfloat16, kind="ExternalInput").ap()
    wr = nc.dram_tensor("wr", [din, fout], mybir.dt.bfloat16, kind="ExternalInput").ap()
    xl_o = nc.dram_tensor("xl", [n_nodes, fout], mybir.dt.bfloat16, kind="ExternalOutput").ap()
    xr_o = nc.dram_tensor("xr", [n_nodes, fout], mybir.dt.bfloat16, kind="ExternalOutput").ap()
    nk = din // P
    nt = n_nodes // P
    with tile.TileContext(nc) as tc, ExitStack() as ctx:
        wpool = ctx.enter_context(tc.tile_pool(name="w", bufs=1))
        pool = ctx.enter_context(tc.tile_pool(name="m", bufs=3))
        psum = ctx.enter_context(tc.tile_pool(name="ps", bufs=2, space="PSUM"))
        wl_sb = wpool.tile([P, nk, fout], mybir.dt.bfloat16)
        wr_sb = wpool.tile([P, nk, fout], mybir.dt.bfloat16)
        for k in range(nk):
            nc.sync.dma_start(wl_sb[:, k, :], wl[k * P:(k + 1) * P, :])
            nc.sync.dma_start(wr_sb[:, k, :], wr[k * P:(k + 1) * P, :])
        for t in range(nt):
            pl = psum.tile([P, fout], mybir.dt.float32, space="PSUM", name="pl")
            pr = psum.tile([P, fout], mybir.dt.float32, space="PSUM", name="pr")
            for k in range(nk):
                xt_sb = pool.tile([P, P], mybir.dt.bfloat16, name="xt_sb")
                nc.sync.dma_start(xt_sb[:], xT[k * P:(k + 1) * P, t * P:(t + 1) * P])
                nc.tensor.matmul(out=pl[:], lhsT=xt_sb[:], rhs=wl_sb[:, k, :],
                                 start=(k == 0), stop=(k == nk - 1))
                nc.tensor.matmul(out=pr[:], lhsT=xt_sb[:], rhs=wr_sb[:, k, :],
                                 start=(k == 0), stop=(k == nk - 1))
            ol = pool.tile([P, fout], mybir.dt.bfloat16, name="ol")
            orr = pool.tile([P, fout], mybir.dt.bfloat16, name="orr")
            nc.scalar.copy(ol[:], pl[:])
            nc.scalar.copy(orr[:], pr[:])
            nc.sync.dma_start(xl_o[t * P:(t + 1) * P, :], ol[:])
            nc.sync.dma_start(xr_o[t * P:(t + 1) * P, :], orr[:])
    nc.compile()
    return nc


def _build_layer1(sched, n_tab, stop_stage=99):
    """Layer-1 edge aggregation: XL table (bf16) + XR core rows -> h (f32)."""
    g_per_core = sched["g_per_core"]
    d_lo, d_hi, d_tot = sched["d_lo"], sched["d_hi"], sched["d_tot"]
    n_nodes = sched["n_pad_nodes"]
    lo_cols = int(8 * d_lo.sum())
    hi_cols = int(8 * d_hi.sum())
    mask_cols = int(d_tot.sum())

    nc = bacc.Bacc("TRN2", target_bir_lowering=False, debug=False, num_devices=NCORES)
    lo_rows = min(SPLIT, n_tab)
    hi_rows = max(n_tab - SPLIT, P)
    t_lo = nc.dram_tensor("t_lo", [lo_rows, P], mybir.dt.bfloat16, kind="ExternalInput").ap()
    t_hi = nc.dram_tensor("t_hi", [hi_rows, P], mybir.dt.bfloat16, kind="ExternalInput").ap()
    xrc = nc.dram_tensor("xrc", [n_nodes, P], mybir.dt.bfloat16, kind="ExternalInput").ap()
    idx_lo = nc.dram_tensor("idx_lo", [P, max(lo_cols, 16)], mybir.dt.int16, kind="ExternalInput").ap()
    idx_hi = nc.dram_tensor("idx_hi", [P, max(hi_cols, 16)], mybir.dt.int16, kind="ExternalInput").ap()
    mask_in = nc.dram_tensor("mask", [P, mask_cols], mybir.dt.bfloat16, kind="ExternalInput").ap()
    att_in = nc.dram_tensor("att", [P, P], mybir.dt.bfloat16, kind="ExternalInput").ap()
    b1_in = nc.dram_tensor("b1", [P, P], mybir.dt.float32, kind="ExternalInput").ap()
    h_out = nc.dram_tensor("h", [n_nodes, P], mybir.dt.float32, kind="ExternalOutput").ap()

    with tile.TileContext(nc) as tc, ExitStack() as ctx:
        cpool = ctx.enter_context(tc.tile_pool(name="c", bufs=1))
        pool = ctx.enter_context(tc.tile_pool(name="m", bufs=2))
        nc.gpsimd.load_library(library_config.mlp)

        idx_lo_sb = cpool.tile([P, max(lo_cols, 16)], mybir.dt.int16)
        idx_hi_sb = cpool.tile([P, max(hi_cols, 16)], mybir.dt.int16)
        mask_sb = cpool.tile([P, mask_cols], mybir.dt.bfloat16)
        att_sb = cpool.tile([P, P], mybir.dt.bfloat16)
        b1_sb = cpool.tile([P, P], mybir.dt.float32)
        nc.sync.dma_start(idx_lo_sb[:], idx_lo[:])
        nc.sync.dma_start(idx_hi_sb[:], idx_hi[:])
        nc.sync.dma_start(mask_sb[:], mask_in[:])
        nc.sync.dma_start(att_sb[:], att_in[:])
        nc.sync.dma_start(b1_sb[:], b1_in[:])

        lo_off = hi_off = m_off = 0
        for g in range(g_per_core):
            dlo, dhi, d = int(d_lo[g]), int(d_hi[g]), int(d_tot[g])
            xl_g = pool.tile([P, d, P], mybir.dt.bfloat16, name="xl_g")
            if dlo:
                nc.gpsimd.dma_gather(
                    xl_g[:, 0:dlo, :], t_lo[:], idx_lo_sb[:, lo_off:lo_off + 8 * dlo],
                    dlo * P, dlo * P, P, single_packet=(dlo * P <= 1024))
            if dhi:
                nc.gpsimd.dma_gather(
                    xl_g[:, dlo:d, :], t_hi[:], idx_hi_sb[:, hi_off:hi_off + 8 * dhi],
                    dhi * P, dhi * P, P, single_packet=(dhi * P <= 1024))
            lo_off += 8 * dlo
            hi_off += 8 * dhi

            xr_sb = pool.tile([P, P], mybir.dt.bfloat16, name="xr_sb")
            nc.sync.dma_start(xr_sb[:], xrc[g * P:(g + 1) * P, :])

            if stop_stage <= 1:
                dbg = pool.tile([P, P], mybir.dt.float32, name="dbg1")
                nc.vector.tensor_copy(out=dbg[:], in_=xl_g[:, 0, :])
                nc.sync.dma_start(h_out[g * P:(g + 1) * P, :], dbg[:])
                continue

            z = pool.tile([P, d, P], mybir.dt.bfloat16, name="z")
            xr_b = xr_sb[:].rearrange("p (o f) -> p o f", o=1).to_broadcast([P, d, P])
            nc.vector.tensor_tensor(out=z[:], in0=xl_g[:], in1=xr_b, op=mybir.AluOpType.add)
            if stop_stage <= 2:
                dbg = pool.tile([P, P], mybir.dt.float32, name="dbg2")
                nc.vector.tensor_copy(out=dbg[:], in_=z[:, 0, :])
                nc.sync.dma_start(h_out[g * P:(g + 1) * P, :], dbg[:])
                continue
            za = pool.tile([P, d, P], mybir.dt.bfloat16, name="za")
            nc.scalar.activation(za[:], z[:], mybir.ActivationFunctionType.Prelu, alpha=NEG_SLOPE)
            if stop_stage <= 3:
                dbg = pool.tile([P, P], mybir.dt.float32, name="dbg3")
                nc.vector.tensor_copy(out=dbg[:], in_=za[:, 0, :])
                nc.sync.dma_start(h_out[g * P:(g + 1) * P, :], dbg[:])
                continue

            tmp = pool.tile([P, d, P], mybir.dt.bfloat16, name="tmp")
            att_b = att_sb[:].rearrange("p (o f) -> p o f", o=1).to_broadcast([P, d, P])
            nc.vector.tensor_tensor(out=tmp[:], in0=za[:], in1=att_b, op=mybir.AluOpType.mult)

            logits = pool.tile([P, d, 8], mybir.dt.float32, name="logits")
            nc.vector.tensor_reduce(
                out=logits[:].rearrange("p s h -> p (s h)"),
                in_=tmp[:].rearrange("p s (h dd) -> p (s h) dd", h=8, dd=16),
                axis=mybir.AxisListType.X, op=mybir.AluOpType.add)

            if stop_stage <= 4:
                dbg = pool.tile([P, P], mybir.dt.float32, name="dbg4")
                nc.gpsimd.memset(dbg[:], 0.0)
                nc.vector.tensor_copy(out=dbg[:, 0:8], in_=logits[:, 0, :])
                nc.sync.dma_start(h_out[g * P:(g + 1) * P, :], dbg[:])
                continue
            ex = pool.tile([P, d, 8], mybir.dt.float32, name="ex")
            nc.scalar.activation(ex[:], logits[:], mybir.ActivationFunctionType.Exp)
            exm = pool.tile([P, d, 8], mybir.dt.bfloat16, name="exm")
            m_b = mask_sb[:, m_off:m_off + d].rearrange("p (s o) -> p s o", o=1).to_broadcast([P, d, 8])
            nc.vector.tensor_tensor(out=exm[:], in0=ex[:], in1=m_b, op=mybir.AluOpType.mult)
            m_off += d

            if stop_stage <= 5:
                dbg = pool.tile([P, P], mybir.dt.float32, name="dbg5")
                nc.gpsimd.memset(dbg[:], 0.0)
                nc.vector.tensor_copy(out=dbg[:, 0:8], in_=exm[:, 0, :])
                nc.sync.dma_start(h_out[g * P:(g + 1) * P, :], dbg[:])
                continue
            denom = pool.tile([P, 8], mybir.dt.float32, name="denom")
            nc.vector.tensor_reduce(
                out=denom[:], in_=exm[:].rearrange("p s h -> p h s"),
                axis=mybir.AxisListType.X, op=mybir.AluOpType.add)
            recip = pool.tile([P, 8], mybir.dt.float32, name="recip")
            nc.vector.tensor_scalar(out=denom[:], in0=denom[:], scalar1=EPS, scalar2=None,
                                    op0=mybir.AluOpType.add)
            nc.vector.reciprocal(recip[:], denom[:])

            if stop_stage <= 6:
                dbg = pool.tile([P, P], mybir.dt.float32, name="dbg6")
                nc.gpsimd.memset(dbg[:], 0.0)
                nc.vector.tensor_copy(out=dbg[:, 0:8], in_=recip[:])
                nc.sync.dma_start(h_out[g * P:(g + 1) * P, :], dbg[:])
                continue
            w2 = pool.tile([P, P, d], mybir.dt.bfloat16, name="w2")
            ex_b = exm[:].rearrange("p s (h o) -> p s h o", o=1).to_broadcast([P, d, 8, 16])
            nc.vector.tensor_tensor(
                out=w2[:].rearrange("p (h dd) k -> p k h dd", h=8, dd=16),
                in0=xl_g[:].rearrange("p s (h dd) -> p s h dd", h=8, dd=16),
                in1=ex_b, op=mybir.AluOpType.mult)

            if stop_stage <= 7:
                dbg = pool.tile([P, P], mybir.dt.float32, name="dbg7")
                nc.vector.tensor_copy(out=dbg[:], in_=w2[:, :, 0])
                nc.sync.dma_start(h_out[g * P:(g + 1) * P, :], dbg[:])
                continue
            u = pool.tile([P, P], mybir.dt.float32, name="u")
            nc.vector.tensor_reduce(out=u[:], in_=w2[:], axis=mybir.AxisListType.X,
                                    op=mybir.AluOpType.add)
            if stop_stage <= 8:
                nc.sync.dma_start(h_out[g * P:(g + 1) * P, :], u[:])
                continue

            # h = elu(u * recip_bcast + b1)
            us = pool.tile([P, P], mybir.dt.float32, name="us")
            r_b = recip[:].rearrange("p (h o) -> p h o", o=1).to_broadcast([P, 8, 16])
            nc.vector.tensor_tensor(out=us[:].rearrange("p (h dd) -> p h dd", h=8, dd=16),
                                    in0=u[:].rearrange("p (h dd) -> p h dd", h=8, dd=16),
                                    in1=r_b, op=mybir.AluOpType.mult)
            nc.vector.tensor_tensor(out=us[:], in0=us[:], in1=b1_sb[:], op=mybir.AluOpType.add)
            relu_t = pool.tile([P, P], mybir.dt.float32, name="relu_t")
            nc.scalar.activation(relu_t[:], us[:], mybir.ActivationFunctionType.Relu)
            exp_t = pool.tile([P, P], mybir.dt.float32, name="exp_t")
            nc.scalar.activation(exp_t[:], us[:], mybir.ActivationFunctionType.Exp)
            nc.vector.tensor_scalar(out=exp_t[:], in0=exp_t[:], scalar1=1.0, scalar2=-1.0,
                                    op0=mybir.AluOpType.min, op1=mybir.AluOpType.add)
            h_t = pool.tile([P, P], mybir.dt.float32, name="h_t")
            nc.vector.tensor_tensor(out=h_t[:], in0=relu_t[:], in1=exp_t[:], op=mybir.AluOpType.add)
            nc.sync.dma_start(h_out[g * P:(g + 1) * P, :], h_t[:])
    nc.compile()
    return nc


def _build_layer2(sched, n_tab, fin, fout):
    """h -> HL/HR transforms + layer-2 edge aggregation + log_softmax.

    hTf [fin, n_tab_pad] bf16 global (for HL table), hTc [fin, n_nodes] bf16
    core nodes (for HR).  fout = 64.
    """
    g_per_core = sched["g_per_core"]
    d_lo, d_hi, d_tot = sched["d_lo"], sched["d_hi"], sched["d_tot"]
    n_nodes = sched["n_pad_nodes"]
    lo_cols = int(8 * d_lo.sum())
    hi_cols = int(8 * d_hi.sum())
    mask_cols = int(d_tot.sum())
    n_tab_pad = -(-n_tab // P) * P

    nc = bacc.Bacc("TRN2", target_bir_lowering=False, debug=False, num_devices=NCORES)
    hTf = nc.dram_tensor("hTf", [fin, n_tab_pad], mybir.dt.bfloat16, kind="ExternalInput").ap()
    hTc = nc.dram_tensor("hTc", [fin, n_nodes], mybir.dt.bfloat16, kind="ExternalInput").ap()
    wl = nc.dram_tensor("wl", [fin, fout], mybir.dt.bfloat16, kind="ExternalInput").ap()
    wr = nc.dram_tensor("wr", [fin, fout], mybir.dt.bfloat16, kind="ExternalInput").ap()
    idx_lo = nc.dram_tensor("idx_lo", [P, max(lo_cols, 16)], mybir.dt.int16, kind="ExternalInput").ap()
    idx_hi = nc.dram_tensor("idx_hi", [P, max(hi_cols, 16)], mybir.dt.int16, kind="ExternalInput").ap()
    mask_in = nc.dram_tensor("mask", [P, mask_cols], mybir.dt.float32, kind="ExternalInput").ap()
    att_in = nc.dram_tensor("att", [P, fout], mybir.dt.float32, kind="ExternalInput").ap()
    b2_in = nc.dram_tensor("b2", [P, fout], mybir.dt.float32, kind="ExternalInput").ap()
    out_o = nc.dram_tensor("out", [n_nodes, fout], mybir.dt.float32, kind="ExternalOutput").ap()

    hl_tab = nc.dram_tensor("hl_tab", [n_tab_pad, fout], mybir.dt.float32, kind="Internal").ap()
    hr_tab = nc.dram_tensor("hr_tab", [n_nodes, fout], mybir.dt.float32, kind="Internal").ap()

    with tile.TileContext(nc) as tc, ExitStack() as ctx:
        cpool = ctx.enter_context(tc.tile_pool(name="c", bufs=1))
        pool = ctx.enter_context(tc.tile_pool(name="m", bufs=2))
        psum = ctx.enter_context(tc.tile_pool(name="ps", bufs=2, space="PSUM"))
        nc.gpsimd.load_library(library_config.mlp)

        wl_sb = cpool.tile([P, fout], mybir.dt.bfloat16)
        wr_sb = cpool.tile([P, fout], mybir.dt.bfloat16)
        nc.sync.dma_start(wl_sb[:], wl[:])
        nc.sync.dma_start(wr_sb[:], wr[:])

        # HL table (global) + HR table (core nodes)
        for t in range(n_tab_pad // P):
            ht = pool.tile([P, P], mybir.dt.bfloat16, name="ht")
            nc.sync.dma_start(ht[:], hTf[:, t * P:(t + 1) * P])
            pmm = psum.tile([P, fout], mybir.dt.float32, space="PSUM", name="pmm")
            nc.tensor.matmul(out=pmm[:], lhsT=ht[:], rhs=wl_sb[:], start=True, stop=True)
            ol = pool.tile([P, fout], mybir.dt.float32, name="ol")
            nc.scalar.copy(ol[:], pmm[:])
            nc.sync.dma_start(hl_tab[t * P:(t + 1) * P, :], ol[:])
        for t in range(n_nodes // P):
            ht2 = pool.tile([P, P], mybir.dt.bfloat16, name="ht2")
            nc.sync.dma_start(ht2[:], hTc[:, t * P:(t + 1) * P])
            pmm2 = psum.tile([P, fout], mybir.dt.float32, space="PSUM", name="pmm2")
            nc.tensor.matmul(out=pmm2[:], lhsT=ht2[:], rhs=wr_sb[:], start=True, stop=True)
            or2 = pool.tile([P, fout], mybir.dt.float32, name="or2")
            nc.scalar.copy(or2[:], pmm2[:])
            nc.sync.dma_start(hr_tab[t * P:(t + 1) * P, :], or2[:])

        idx_lo_sb = cpool.tile([P, max(lo_cols, 16)], mybir.dt.int16)
        idx_hi_sb = cpool.tile([P, max(hi_cols, 16)], mybir.dt.int16)
        mask_sb = cpool.tile([P, mask_cols], mybir.dt.float32)
        att_sb = cpool.tile([P, fout], mybir.dt.float32)
        b2_sb = cpool.tile([P, fout], mybir.dt.float32)
        nc.sync.dma_start(idx_lo_sb[:], idx_lo[:])
        nc.sync.dma_start(idx_hi_sb[:], idx_hi[:])
        nc.sync.dma_start(mask_sb[:], mask_in[:])
        nc.sync.dma_start(att_sb[:], att_in[:])
        nc.sync.dma_start(b2_sb[:], b2_in[:])

        lo_off = hi_off = m_off = 0
        for g in range(g_per_core):
            dlo, dhi, d = int(d_lo[g]), int(d_hi[g]), int(d_tot[g])
            xl_g = pool.tile([P, d, fout], mybir.dt.float32, name="xl_g")
            if dlo:
                nc.gpsimd.dma_gather(
                    xl_g[:, 0:dlo, :], hl_tab[0:min(SPLIT, n_tab_pad), :],
                    idx_lo_sb[:, lo_off:lo_off + 8 * dlo],
                    dlo * P, dlo * P, fout, single_packet=(dlo * P <= 1024))
            if dhi:
                nc.gpsimd.dma_gather(
                    xl_g[:, dlo:d, :], hl_tab[SPLIT:n_tab_pad, :], idx_hi_sb[:, hi_off:hi_off + 8 * dhi],
                    dhi * P, dhi * P, fout, single_packet=(dhi * P <= 1024))
            lo_off += 8 * dlo
            hi_off += 8 * dhi

            xr_sb = pool.tile([P, fout], mybir.dt.float32, name="xr_sb")
            nc.sync.dma_start(xr_sb[:], hr_tab[g * P:(g + 1) * P, :])

            z = pool.tile([P, d, fout], mybir.dt.float32, name="z")
            xr_b = xr_sb[:].rearrange("p (o f) -> p o f", o=1).to_broadcast([P, d, fout])
            nc.vector.tensor_tensor(out=z[:], in0=xl_g[:], in1=xr_b, op=mybir.AluOpType.add)
            za = pool.tile([P, d, fout], mybir.dt.float32, name="za")
            nc.scalar.activation(za[:], z[:], mybir.ActivationFunctionType.Prelu, alpha=NEG_SLOPE)

            tmp = pool.tile([P, d, fout], mybir.dt.float32, name="tmp")
            att_b = att_sb[:].rearrange("p (o f) -> p o f", o=1).to_broadcast([P, d, fout])
            nc.vector.tensor_tensor(out=tmp[:], in0=za[:], in1=att_b, op=mybir.AluOpType.mult)

            logits = pool.tile([P, d], mybir.dt.float32, name="logits")
            nc.vector.tensor_reduce(out=logits[:], in_=tmp[:],
                                    axis=mybir.AxisListType.X, op=mybir.AluOpType.add)

            ex = pool.tile([P, d], mybir.dt.float32, name="ex")
            nc.scalar.activation(ex[:], logits[:], mybir.ActivationFunctionType.Exp)
            exm = pool.tile([P, d], mybir.dt.float32, name="exm")
            nc.vector.tensor_tensor(out=exm[:], in0=ex[:], in1=mask_sb[:, m_off:m_off + d],
                                    op=mybir.AluOpType.mult)
            m_off += d

            denom = pool.tile([P, 1], mybir.dt.float32, name="denom")
            nc.vector.tensor_reduce(out=denom[:], in_=exm[:],
                                    axis=mybir.AxisListType.X, op=mybir.AluOpType.add)
            nc.vector.tensor_scalar(out=denom[:], in0=denom[:], scalar1=EPS, scalar2=None,
                                    op0=mybir.AluOpType.add)
            recip = pool.tile([P, 1], mybir.dt.float32, name="recip")
            nc.vector.reciprocal(recip[:], denom[:])

            w2 = pool.tile([P, fout, d], mybir.dt.float32, name="w2")
            ex_b = exm[:].rearrange("p (s o) -> p s o", o=1).to_broadcast([P, d, fout])
            nc.vector.tensor_tensor(
                out=w2[:].rearrange("p f k -> p k f"),
                in0=xl_g[:], in1=ex_b, op=mybir.AluOpType.mult)

            u = pool.tile([P, fout], mybir.dt.float32, name="u")
            nc.vector.tensor_reduce(out=u[:], in_=w2[:], axis=mybir.AxisListType.X,
                                    op=mybir.AluOpType.add)

            o_t = pool.tile([P, fout], mybir.dt.float32, name="o_t")
            nc.vector.tensor_scalar(out=o_t[:], in0=u[:], scalar1=recip[:, 0:1], scalar2=None,
                                    op0=mybir.AluOpType.mult)
            nc.vector.tensor_tensor(out=o_t[:], in0=o_t[:], in1=b2_sb[:], op=mybir.AluOpType.add)

            # log_softmax over fout
            mx = pool.tile([P, 1], mybir.dt.float32, name="mx")
            nc.vector.tensor_reduce(out=mx[:], in_=o_t[:], axis=mybir.AxisListType.X,
                                    op=mybir.AluOpType.max)
            nmx = pool.tile([P, 1], mybir.dt.float32, name="nmx")
            nc.vector.tensor_scalar(out=nmx[:], in0=mx[:], scalar1=-1.0, scalar2=None,
                                    op0=mybir.AluOpType.mult)
            eo = pool.tile([P, fout], mybir.dt.float32, name="eo")
            se = pool.tile([P, 1], mybir.dt.float32, name="se")
            nc.scalar.activation(eo[:], o_t[:], mybir.ActivationFunctionType.Exp,
                                 bias=nmx[:, 0:1], accum_out=se[:])
            ls = pool.tile([P, 1], mybir.dt.float32, name="ls")
            nc.scalar.activation(ls[:], se[:], mybir.ActivationFunctionType.Ln)
            shift = pool.tile([P, 1], mybir.dt.float32, name="shift")
            nc.vector.tensor_tensor(out=shift[:], in0=mx[:], in1=ls[:], op=mybir.AluOpType.add)
            res = pool.tile([P, fout], mybir.dt.float32, name="res")
            nc.vector.tensor_scalar(out=res[:], in0=o_t[:], scalar1=shift[:, 0:1], scalar2=None,
                                    op0=mybir.AluOpType.subtract)
            nc.sync.dma_start(out_o[g * P:(g + 1) * P, :], res[:])
    nc.compile()
    return nc


# --------------------------------------------------------------------------
# top-level
# --------------------------------------------------------------------------

def kernel(x, edge_index, Wl1, Wr1, att1, b1, Wl2, Wr2, att2, b2):
    n, din = x.shape
    f1 = Wl1.shape[1]          # 128
    f2 = Wl2.shape[1]          # 64
    heads, dh = att1.shape     # 8, 16

    sched = _prep_graph(np.asarray(edge_index, dtype=np.int64), n)
    node_list = sched["node_list"]
    n_core = sched["n_pad_nodes"]
    g_per_core = sched["g_per_core"]

    x_np = np.asarray(x, dtype=np.float32)

    # ---------------- launch A: x -> XL, XR (per-core nodes) ----------------
    nid_safe = [np.maximum(node_list[c], 0) for c in range(NCORES)]
    valid = [(node_list[c] >= 0) for c in range(NCORES)]
    ncA = _build_transform1(n_core, din, f1)
    wl1b = Wl1.astype(BF16)
    wr1b = Wr1.astype(BF16)
    in_maps = []
    for c in range(NCORES):
        xc = x_np[nid_safe[c]] * valid[c][:, None]
        in_maps.append({
            "xT": np.ascontiguousarray(xc.T.astype(BF16)),
            "wl": wl1b, "wr": wr1b,
        })
    resA = _run(ncA, in_maps)

    # assemble global XL table (bf16); XR stays per-core
    xl_tab = np.zeros((n, f1), dtype=BF16)
    xr_core = []
    for c in range(NCORES):
        xl_tab[node_list[c][valid[c]]] = resA[c]["xl"][valid[c]]
        xr_core.append(resA[c]["xr"])

    # ---------------- launch B: layer-1 edges -> h --------------------------
    ncB = _build_layer1(sched, n)
    att_tile = np.tile(att1.reshape(1, heads * dh).astype(BF16), (P, 1))
    b1_tile = np.tile(b1.reshape(1, f1).astype(np.float32), (P, 1))
    t_lo = np.ascontiguousarray(xl_tab[:min(SPLIT, n)])
    hi_rows = max(n - SPLIT, P)
    t_hi = np.zeros((hi_rows, f1), dtype=BF16)
    if n > SPLIT:
        t_hi[:n - SPLIT] = xl_tab[SPLIT:]
    in_maps = []
    for c in range(NCORES):
        in_maps.append({
            "t_lo": t_lo, "t_hi": t_hi,
            "xrc": xr_core[c],
            "idx_lo": _pad_cols(_concat_idx(sched["idx_lo"][c]), 16),
            "idx_hi": _pad_cols(_concat_idx(sched["idx_hi"][c]), 16),
            "mask": np.ascontiguousarray(
                np.concatenate(sched["masks"][c], axis=1).astype(BF16)),
            "att": att_tile, "b1": b1_tile,
        })
    resB = _run(ncB, in_maps)

    # assemble h (f32) -> hT tables
    h_full = np.zeros((n, f1), dtype=np.float32)
    for c in range(NCORES):
        h_full[node_list[c][valid[c]]] = resB[c]["h"][valid[c]]
    n_tab_pad = -(-n // P) * P
    hT_full = np.zeros((f1, n_tab_pad), dtype=BF16)
    hT_full[:, :n] = h_full.T.astype(BF16)

    # ---------------- launch C: layer-2 + log_softmax -----------------------
    ncC = _build_layer2(sched, n, f1, f2)
    att2_tile = np.tile(att2.reshape(1, f2).astype(np.float32), (P, 1))
    b2_tile = np.tile(b2.reshape(1, f2).astype(np.float32), (P, 1))
    in_maps = []
    for c in range(NCORES):
        hc = h_full[nid_safe[c]] * valid[c][:, None]
        in_maps.append({
            "hTf": hT_full,
            "hTc": np.ascontiguousarray(hc.T.astype(BF16)),
            "wl": Wl2.astype(BF16), "wr": Wr2.astype(BF16),
            "idx_lo": _pad_cols(_concat_idx(sched["idx_lo"][c]), 16),
            "idx_hi": _pad_cols(_concat_idx(sched["idx_hi"][c]), 16),
            "mask": np.ascontiguousarray(
                np.concatenate(sched["masks"][c], axis=1).astype(np.float32)),
            "att": att2_tile, "b2": b2_tile,
        })
    resC = _run(ncC, in_maps)

    out = np.zeros((n, f2), dtype=np.float32)
    for c in range(NCORES):
        out[node_list[c][valid[c]]] = resC[c]["out"][valid[c]]
    return out


def _pad_cols(arr, want):
    cols = arr.shape[1]
    want = max(want, cols)
    if cols == want:
        return np.ascontiguousarray(arr)
    out = np.zeros((arr.shape[0], want), dtype=arr.dtype)
    out[:, :cols] = arr
    return out


# revision 11
# speedup vs baseline: 14.0852x; 14.0852x over previous
"""Trainium2 Bass kernel for 2-layer GATv2 (nn_GAT_25958782337775).

Strategy (8 NeuronCores, SPMD):
  - Nodes are sorted by (deg_lo, deg_hi) of their incoming edges, grouped into
    128-node groups of near-uniform in-degree, and dealt round-robin to the 8
    cores (destination-node sharding per the hint).
  - Per group, incoming edges live in dense [128 nodes, D slots] tensors:
    source features are fetched with dma_gather (bf16 rows), the
    segment-softmax + weighted sum become plain free-dim reductions, and
    invalid slots are masked after exp.  No scatter is ever needed.
  - 3 device launches: (A) per-core feature transforms x->XL,XR;
    (B) layer-1 edge aggregation -> h; (C) h->HL,HR transforms + layer-2
    edge aggregation + log_softmax.  The host only reshards/casts between
    launches (index bookkeeping; all FLOPs happen on device).

Numerics: gather tables bf16; accumulations f32.  Softmax is computed without
the max-subtraction (logits are O(1); identical in exact arithmetic, matches
the reference to fp rounding).  log_softmax does use the max shift.
"""

import sys

sys.path.insert(0, "/opt/trn_rl_repo")

from contextlib import ExitStack

import numpy as np
import ml_dtypes

import concourse.bacc as bacc
import concourse.tile as tile
from concourse import mybir, library_config
from concourse.bass_utils import run_bass_kernel_spmd

BF16 = ml_dtypes.bfloat16
NCORES = 8
SPLIT = 32768
P = 128
EPS = 1e-16
NEG_SLOPE = 0.2
RING = 131072  # SWDGE descriptor-ring carveout (bytes); big for gather overlap

_TRACE = {"on": False, "exec_ns": 0.0}


# --------------------------------------------------------------------------
# host-side graph preprocessing
# --------------------------------------------------------------------------

def _prep_graph(edge_index: np.ndarray, n: int):
    """Group nodes by in-degree, build per-core dense gather schedules."""
    src = np.concatenate([edge_index[0], np.arange(n, dtype=np.int64)])
    dst = np.concatenate([edge_index[1], np.arange(n, dtype=np.int64)])

    comb = np.lexsort(((src >= SPLIT).astype(np.int64), dst))
    src_s = src[comb]
    dst_s = dst[comb]

    deg = np.bincount(dst_s, minlength=n)
    deg_lo = np.bincount(dst_s[src_s < SPLIT], minlength=n)
    deg_hi = deg - deg_lo
    starts = np.zeros(n + 1, dtype=np.int64)
    np.cumsum(deg, out=starts[1:])

    node_order = np.lexsort((deg_hi, deg_lo))  # ascending degree
    ng = -(-n // P) // NCORES * NCORES
    while ng * P < n:
        ng += NCORES
    n_pad = ng * P
    padded = np.concatenate([np.full(n_pad - n, -1, dtype=np.int64), node_order])

    g_per_core = ng // NCORES
    node_list = np.zeros((NCORES, g_per_core * P), dtype=np.int64)
    for j in range(ng):
        c, s = j % NCORES, j // NCORES
        node_list[c, s * P:(s + 1) * P] = padded[j * P:(j + 1) * P]

    d_lo = np.zeros(g_per_core, dtype=np.int64)
    d_hi = np.zeros(g_per_core, dtype=np.int64)
    for s in range(g_per_core):
        ids = node_list[:, s * P:(s + 1) * P].reshape(-1)
        real = ids[ids >= 0]
        if len(real):
            d_lo[s] = deg_lo[real].max()
            d_hi[s] = deg_hi[real].max()
    d_tot = d_lo + d_hi

    idx_lo_cores, idx_hi_cores, mask_cores = [], [], []
    for c in range(NCORES):
        lo_blocks, hi_blocks, masks = [], [], []
        for s in range(g_per_core):
            ids = node_list[c, s * P:(s + 1) * P]
            dlo, dhi = int(d_lo[s]), int(d_hi[s])
            lo_idx = np.zeros((dlo, P), dtype=np.int64)
            hi_idx = np.zeros((dhi, P), dtype=np.int64)
            m = np.zeros((P, dlo + dhi), dtype=np.float32)
            for p in range(P):
                nid = ids[p]
                if nid < 0:
                    continue
                a, b = starts[nid], starts[nid + 1]
                es = src_s[a:b]
                nlo = int(deg_lo[nid])
                lo_idx[:nlo, p] = es[:nlo]
                hi_idx[: int(deg_hi[nid]), p] = es[nlo:] - SPLIT
                m[p, :nlo] = 1.0
                m[p, dlo:dlo + int(deg_hi[nid])] = 1.0
            lo_blocks.append(lo_idx.reshape(-1))
            hi_blocks.append(hi_idx.reshape(-1))
            masks.append(m)
        idx_lo_cores.append(lo_blocks)
        idx_hi_cores.append(hi_blocks)
        mask_cores.append(masks)

    return dict(
        node_list=node_list, d_lo=d_lo, d_hi=d_hi, d_tot=d_tot,
        g_per_core=g_per_core, n_pad_nodes=g_per_core * P,
        idx_lo=idx_lo_cores, idx_hi=idx_hi_cores, masks=mask_cores,
    )


def _wrap_idx(flat: np.ndarray) -> np.ndarray:
    assert len(flat) % 16 == 0
    arr = flat.astype(np.int16).reshape(-1, 16).T
    return np.tile(arr, (8, 1))


def _concat_idx(blocks):
    cols = [_wrap_idx(b) if len(b) else np.zeros((128, 0), np.int16) for b in blocks]
    return np.ascontiguousarray(np.concatenate(cols, axis=1)) if cols else np.zeros((128, 0), np.int16)


def _pad_cols(arr, want):
    cols = arr.shape[1]
    want = max(want, cols)
    if cols == want:
        return np.ascontiguousarray(arr)
    out = np.zeros((arr.shape[0], want), dtype=arr.dtype)
    out[:, :cols] = arr
    return out


# --------------------------------------------------------------------------
# launch builders
# --------------------------------------------------------------------------

def _run(nc, in_maps):
    if _TRACE["on"]:
        # No NTFF capture in this container; use the instruction cost model
        # (TimelineSim) for the per-launch device-time estimate.
        from concourse.timeline_sim import TimelineSim
        tl = TimelineSim(nc, trace=False)
        ns = tl.simulate()
        _TRACE["exec_ns"] += ns
        _TRACE.setdefault("per_launch", []).append(ns)
    res = run_bass_kernel_spmd(
        nc, in_maps, core_ids=list(range(NCORES)), trace=False,
    )
    return res.results


def _new_nc():
    return bacc.Bacc(
        "TRN2", target_bir_lowering=False, debug=False, num_devices=NCORES,
        dynamic_dma_scratch_size=RING,
    )


def _build_transform1(n_nodes, din, fout):
    """XL = x @ Wl, XR = x @ Wr for this core's nodes (xT [din, n_nodes] bf16).

    Slab-batched: per slab of SLAB node-tiles, one big xT DMA, matmuls into a
    single PSUM bank per output, one wide psum->sbuf copy, one wide store.
    """
    SLAB = 4
    nc = _new_nc()
    xT = nc.dram_tensor("xT", [din, n_nodes], mybir.dt.bfloat16, kind="ExternalInput").ap()
    wl = nc.dram_tensor("wl", [din, fout], mybir.dt.bfloat16, kind="ExternalInput").ap()
    wr = nc.dram_tensor("wr", [din, fout], mybir.dt.bfloat16, kind="ExternalInput").ap()
    xl_o = nc.dram_tensor("xl", [n_nodes, fout], mybir.dt.bfloat16, kind="ExternalOutput").ap()
    xr_o = nc.dram_tensor("xr", [n_nodes, fout], mybir.dt.bfloat16, kind="ExternalOutput").ap()
    nk = din // P
    nt = n_nodes // P
    assert nt % SLAB == 0
    xT_v = xT.rearrange("(k p) n -> p k n", p=P)  # [P, nk, n_nodes]
    with tile.TileContext(nc) as tc, ExitStack() as ctx:
        wpool = ctx.enter_context(tc.tile_pool(name="w", bufs=1))
        pool = ctx.enter_context(tc.tile_pool(name="m", bufs=3))
        psum = ctx.enter_context(tc.tile_pool(name="ps", bufs=2, space="PSUM"))
        wl_sb = wpool.tile([P, nk, fout], mybir.dt.bfloat16)
        wr_sb = wpool.tile([P, nk, fout], mybir.dt.bfloat16)
        nc.sync.dma_start(wl_sb[:], wl.rearrange("(k p) f -> p k f", p=P))
        nc.sync.dma_start(wr_sb[:], wr.rearrange("(k p) f -> p k f", p=P))
        for s in range(nt // SLAB):
            xt_sb = pool.tile([P, nk, SLAB * P], mybir.dt.bfloat16, name="xt_sb")
            nc.sync.dma_start(xt_sb[:], xT_v[:, :, s * SLAB * P:(s + 1) * SLAB * P])
            pl = psum.tile([P, SLAB * fout], mybir.dt.float32, space="PSUM", name="pl")
            pr = psum.tile([P, SLAB * fout], mybir.dt.float32, space="PSUM", name="pr")
            for t in range(SLAB):
                for k in range(nk):
                    lhsT = xt_sb[:, k, t * P:(t + 1) * P]
                    nc.tensor.matmul(out=pl[:, t * fout:(t + 1) * fout], lhsT=lhsT,
                                     rhs=wl_sb[:, k, :], start=(k == 0), stop=(k == nk - 1))
                    nc.tensor.matmul(out=pr[:, t * fout:(t + 1) * fout], lhsT=lhsT,
                                     rhs=wr_sb[:, k, :], start=(k == 0), stop=(k == nk - 1))
            ol = pool.tile([P, SLAB * fout], mybir.dt.bfloat16, name="ol")
            orr = pool.tile([P, SLAB * fout], mybir.dt.bfloat16, name="orr")
            nc.scalar.copy(ol[:], pl[:])
            nc.scalar.copy(orr[:], pr[:])
            nc.sync.dma_start(
                xl_o[s * SLAB * P:(s + 1) * SLAB * P, :].rearrange("(t p) f -> p t f", p=P),
                ol[:].rearrange("p (t f) -> p t f", t=SLAB))
            nc.sync.dma_start(
                xr_o[s * SLAB * P:(s + 1) * SLAB * P, :].rearrange("(t p) f -> p t f", p=P),
                orr[:].rearrange("p (t f) -> p t f", t=SLAB))
    nc.compile()
    return nc


def _edge_phase(nc, pool, sched, params):
    """Shared edge-aggregation loop (both layers).  See callers for params."""
    g_per_core = sched["g_per_core"]
    d_lo, d_hi, d_tot = sched["d_lo"], sched["d_hi"], sched["d_tot"]
    fd, heads, dh = params["fd"], params["heads"], params["dh"]
    t_lo, t_hi = params["t_lo"], params["t_hi"]
    xr_tab = params["xr_tab"]
    idx_lo_sb, idx_hi_sb = params["idx_lo_sb"], params["idx_hi_sb"]
    mask_sb, att_sb = params["mask_sb"], params["att_sb"]
    emit = params["emit"]

    lo_off = hi_off = m_off = 0
    for g in range(g_per_core):
        dlo, dhi, d = int(d_lo[g]), int(d_hi[g]), int(d_tot[g])
        if d == 0:
            continue
        xl_g = pool.tile([P, d, P], mybir.dt.bfloat16, name="xl_g")
        if dlo:
            nc.gpsimd.dma_gather(
                xl_g[:, 0:dlo, :], t_lo, idx_lo_sb[:, lo_off:lo_off + 8 * dlo],
                dlo * P, dlo * P, P, single_packet=(dlo * P <= 1024))
        if dhi:
            nc.gpsimd.dma_gather(
                xl_g[:, dlo:d, :], t_hi, idx_hi_sb[:, hi_off:hi_off + 8 * dhi],
                dhi * P, dhi * P, P, single_packet=(dhi * P <= 1024))
        lo_off += 8 * dlo
        hi_off += 8 * dhi

        xr_sb = pool.tile([P, fd], mybir.dt.bfloat16, name="xr_sb")
        nc.sync.dma_start(xr_sb[:], xr_tab[g * P:(g + 1) * P, 0:fd])

        xl_c = xl_g[:, :, 0:fd] if fd < P else xl_g[:]

        z = pool.tile([P, d, fd], mybir.dt.bfloat16, name="z")
        xr_b = xr_sb[:].rearrange("p (o f) -> p o f", o=1).to_broadcast([P, d, fd])
        nc.vector.tensor_tensor(out=z[:], in0=xl_c, in1=xr_b, op=mybir.AluOpType.add)
        za = pool.tile([P, d, fd], mybir.dt.bfloat16, name="za")
        nc.scalar.activation(za[:], z[:], mybir.ActivationFunctionType.Prelu, alpha=NEG_SLOPE)

        # att-mul reuses z (dead after prelu)
        att_b = att_sb[:].rearrange("p (o f) -> p o f", o=1).to_broadcast([P, d, fd])
        nc.vector.tensor_tensor(out=z[:], in0=za[:], in1=att_b, op=mybir.AluOpType.mult)

        logits = pool.tile([P, d, heads], mybir.dt.float32, name="logits")
        nc.vector.tensor_reduce(
            out=logits[:].rearrange("p s h -> p (s h)"),
            in_=z[:].rearrange("p s (h dd) -> p (s h) dd", h=heads, dd=dh),
            axis=mybir.AxisListType.X, op=mybir.AluOpType.add)

        ex = pool.tile([P, d, heads], mybir.dt.float32, name="ex")
        nc.scalar.activation(ex[:], logits[:], mybir.ActivationFunctionType.Exp)
        exm = pool.tile([P, d, heads], mybir.dt.bfloat16, name="exm")
        m_b = mask_sb[:, m_off:m_off + d].rearrange("p (s o) -> p s o", o=1).to_broadcast([P, d, heads])
        nc.vector.tensor_tensor(out=exm[:], in0=ex[:], in1=m_b, op=mybir.AluOpType.mult)
        m_off += d

        denom = pool.tile([P, heads], mybir.dt.float32, name="denom")
        nc.vector.tensor_reduce(
            out=denom[:], in_=exm[:].rearrange("p s h -> p h s"),
            axis=mybir.AxisListType.X, op=mybir.AluOpType.add)
        nc.vector.tensor_scalar(out=denom[:], in0=denom[:], scalar1=EPS, scalar2=None,
                                op0=mybir.AluOpType.add)
        recip = pool.tile([P, heads], mybir.dt.float32, name="recip")
        nc.vector.reciprocal(recip[:], denom[:])

        # expand exm over the head dim on the scalar engine (ACT); reuse z
        exm_b = exm[:].rearrange("p s (h o) -> p s h o", o=1).to_broadcast([P, d, heads, dh])
        nc.scalar.copy(
            z[:].rearrange("p s (h dd) -> p s h dd", h=heads, dd=dh), exm_b)

        # w2 reuses za (dead after att-mul)
        nc.vector.tensor_tensor(out=za[:], in0=xl_c, in1=z[:], op=mybir.AluOpType.mult)

        u = pool.tile([P, fd], mybir.dt.float32, name="u")
        nc.vector.tensor_reduce(out=u[:], in_=za[:].rearrange("p k f -> p f k"),
                                axis=mybir.AxisListType.X, op=mybir.AluOpType.add)

        emit(g, u, recip, pool)


def _build_layer1(sched, n_tab):
    """Layer-1 edge aggregation: XL table (bf16) + XR core rows -> h (f32)."""
    d_lo, d_hi, d_tot = sched["d_lo"], sched["d_hi"], sched["d_tot"]
    n_nodes = sched["n_pad_nodes"]
    lo_cols = int(8 * d_lo.sum())
    hi_cols = int(8 * d_hi.sum())
    mask_cols = int(d_tot.sum())

    nc = _new_nc()
    lo_rows = min(SPLIT, n_tab)
    hi_rows = max(n_tab - SPLIT, P)
    t_lo = nc.dram_tensor("t_lo", [lo_rows, P], mybir.dt.bfloat16, kind="ExternalInput").ap()
    t_hi = nc.dram_tensor("t_hi", [hi_rows, P], mybir.dt.bfloat16, kind="ExternalInput").ap()
    xrc = nc.dram_tensor("xrc", [n_nodes, P], mybir.dt.bfloat16, kind="ExternalInput").ap()
    idx_lo = nc.dram_tensor("idx_lo", [P, max(lo_cols, 16)], mybir.dt.int16, kind="ExternalInput").ap()
    idx_hi = nc.dram_tensor("idx_hi", [P, max(hi_cols, 16)], mybir.dt.int16, kind="ExternalInput").ap()
    mask_in = nc.dram_tensor("mask", [P, mask_cols], mybir.dt.bfloat16, kind="ExternalInput").ap()
    att_in = nc.dram_tensor("att", [P, P], mybir.dt.bfloat16, kind="ExternalInput").ap()
    b1_in = nc.dram_tensor("b1", [P, P], mybir.dt.float32, kind="ExternalInput").ap()
    h_out = nc.dram_tensor("h", [n_nodes, P], mybir.dt.float32, kind="ExternalOutput").ap()

    with tile.TileContext(nc) as tc, ExitStack() as ctx:
        cpool = ctx.enter_context(tc.tile_pool(name="c", bufs=1))
        pool = ctx.enter_context(tc.tile_pool(name="m", bufs=2))
        nc.gpsimd.load_library(library_config.mlp)

        idx_lo_sb = cpool.tile([P, max(lo_cols, 16)], mybir.dt.int16)
        idx_hi_sb = cpool.tile([P, max(hi_cols, 16)], mybir.dt.int16)
        mask_sb = cpool.tile([P, mask_cols], mybir.dt.bfloat16)
        att_sb = cpool.tile([P, P], mybir.dt.bfloat16)
        b1_sb = cpool.tile([P, P], mybir.dt.float32)
        nc.sync.dma_start(idx_lo_sb[:], idx_lo[:])
        nc.sync.dma_start(idx_hi_sb[:], idx_hi[:])
        nc.sync.dma_start(mask_sb[:], mask_in[:])
        nc.sync.dma_start(att_sb[:], att_in[:])
        nc.sync.dma_start(b1_sb[:], b1_in[:])

        def emit(g, u, recip, pool):
            # h = elu(u * recip_bcast + b1)
            us = pool.tile([P, P], mybir.dt.float32, name="us")
            r_b = recip[:].rearrange("p (h o) -> p h o", o=1).to_broadcast([P, 8, 16])
            nc.vector.tensor_tensor(out=us[:].rearrange("p (h dd) -> p h dd", h=8, dd=16),
                                    in0=u[:].rearrange("p (h dd) -> p h dd", h=8, dd=16),
                                    in1=r_b, op=mybir.AluOpType.mult)
            nc.vector.tensor_tensor(out=us[:], in0=us[:], in1=b1_sb[:], op=mybir.AluOpType.add)
            relu_t = pool.tile([P, P], mybir.dt.float32, name="relu_t")
            nc.scalar.activation(relu_t[:], us[:], mybir.ActivationFunctionType.Relu)
            exp_t = pool.tile([P, P], mybir.dt.float32, name="exp_t")
            nc.scalar.activation(exp_t[:], us[:], mybir.ActivationFunctionType.Exp)
            nc.vector.tensor_scalar(out=exp_t[:], in0=exp_t[:], scalar1=1.0, scalar2=-1.0,
                                    op0=mybir.AluOpType.min, op1=mybir.AluOpType.add)
            h_t = pool.tile([P, P], mybir.dt.float32, name="h_t")
            nc.vector.tensor_tensor(out=h_t[:], in0=relu_t[:], in1=exp_t[:], op=mybir.AluOpType.add)
            nc.sync.dma_start(h_out[g * P:(g + 1) * P, :], h_t[:])

        _edge_phase(nc, pool, sched, dict(
            fd=P, heads=8, dh=16, t_lo=t_lo[:], t_hi=t_hi[:], xr_tab=xrc,
            idx_lo_sb=idx_lo_sb, idx_hi_sb=idx_hi_sb, mask_sb=mask_sb,
            att_sb=att_sb, emit=emit))
    nc.compile()
    return nc


def _build_layer2(sched, n_tab, fin, fout):
    """h -> HL/HR transforms + layer-2 edge aggregation + log_softmax.

    hTf [fin, n_tab_pad] bf16 global; hTc [fin, n_nodes] bf16 core nodes.
    HL/HR are built as [*, 128] bf16 with zero right-half (weights are
    host-padded to 128 columns), so gather rows are 256B and the edge phase
    runs on bf16 64-col slices.
    """
    SLAB = 4
    d_lo, d_hi, d_tot = sched["d_lo"], sched["d_hi"], sched["d_tot"]
    n_nodes = sched["n_pad_nodes"]
    lo_cols = int(8 * d_lo.sum())
    hi_cols = int(8 * d_hi.sum())
    mask_cols = int(d_tot.sum())
    n_tab_pad = -(-n_tab // (SLAB * P)) * (SLAB * P)

    nc = _new_nc()
    hTf = nc.dram_tensor("hTf", [fin, n_tab_pad], mybir.dt.bfloat16, kind="ExternalInput").ap()
    hTc = nc.dram_tensor("hTc", [fin, n_nodes], mybir.dt.bfloat16, kind="ExternalInput").ap()
    wl = nc.dram_tensor("wl", [fin, P], mybir.dt.bfloat16, kind="ExternalInput").ap()
    wr = nc.dram_tensor("wr", [fin, P], mybir.dt.bfloat16, kind="ExternalInput").ap()
    idx_lo = nc.dram_tensor("idx_lo", [P, max(lo_cols, 16)], mybir.dt.int16, kind="ExternalInput").ap()
    idx_hi = nc.dram_tensor("idx_hi", [P, max(hi_cols, 16)], mybir.dt.int16, kind="ExternalInput").ap()
    mask_in = nc.dram_tensor("mask", [P, mask_cols], mybir.dt.bfloat16, kind="ExternalInput").ap()
    att_in = nc.dram_tensor("att", [P, fout], mybir.dt.bfloat16, kind="ExternalInput").ap()
    b2_in = nc.dram_tensor("b2", [P, fout], mybir.dt.float32, kind="ExternalInput").ap()
    out_o = nc.dram_tensor("out", [n_nodes, fout], mybir.dt.float32, kind="ExternalOutput").ap()

    hl_tab = nc.dram_tensor("hl_tab", [n_tab_pad, P], mybir.dt.bfloat16, kind="Internal").ap()
    hr_tab = nc.dram_tensor("hr_tab", [n_nodes, P], mybir.dt.bfloat16, kind="Internal").ap()

    with tile.TileContext(nc) as tc, ExitStack() as ctx:
        cpool = ctx.enter_context(tc.tile_pool(name="c", bufs=1))
        pool = ctx.enter_context(tc.tile_pool(name="m", bufs=2))
        psum = ctx.enter_context(tc.tile_pool(name="ps", bufs=2, space="PSUM"))
        nc.gpsimd.load_library(library_config.mlp)

        wl_sb = cpool.tile([P, P], mybir.dt.bfloat16)
        wr_sb = cpool.tile([P, P], mybir.dt.bfloat16)
        nc.sync.dma_start(wl_sb[:], wl[:])
        nc.sync.dma_start(wr_sb[:], wr[:])

        def transform(src_v, dst_tab, w_sb, ntiles, tag):
            assert ntiles % SLAB == 0
            for s in range(ntiles // SLAB):
                ht = pool.tile([P, SLAB * P], mybir.dt.bfloat16, name=f"ht_{tag}")
                nc.sync.dma_start(ht[:], src_v[:, s * SLAB * P:(s + 1) * SLAB * P])
                pm = psum.tile([P, SLAB * P], mybir.dt.float32, space="PSUM", name=f"pm_{tag}")
                for t in range(SLAB):
                    nc.tensor.matmul(out=pm[:, t * P:(t + 1) * P],
                                     lhsT=ht[:, t * P:(t + 1) * P], rhs=w_sb[:],
                                     start=True, stop=True)
                ob = pool.tile([P, SLAB * P], mybir.dt.bfloat16, name=f"ob_{tag}")
                nc.scalar.copy(ob[:], pm[:])
                nc.sync.dma_start(
                    dst_tab[s * SLAB * P:(s + 1) * SLAB * P, :].rearrange("(t p) f -> p t f", p=P),
                    ob[:].rearrange("p (t f) -> p t f", t=SLAB))

        transform(hTf, hl_tab, wl_sb, n_tab_pad // P, "hl")
        transform(hTc, hr_tab, wr_sb, n_nodes // P, "hr")

        idx_lo_sb = cpool.tile([P, max(lo_cols, 16)], mybir.dt.int16)
        idx_hi_sb = cpool.tile([P, max(hi_cols, 16)], mybir.dt.int16)
        mask_sb = cpool.tile([P, mask_cols], mybir.dt.bfloat16)
        att_sb = cpool.tile([P, fout], mybir.dt.bfloat16)
        b2_sb = cpool.tile([P, fout], mybir.dt.float32)
        nc.sync.dma_start(idx_lo_sb[:], idx_lo[:])
        nc.sync.dma_start(idx_hi_sb[:], idx_hi[:])
        nc.sync.dma_start(mask_sb[:], mask_in[:])
        nc.sync.dma_start(att_sb[:], att_in[:])
        nc.sync.dma_start(b2_sb[:], b2_in[:])

        def emit(g, u, recip, pool):
            o_t = pool.tile([P, fout], mybir.dt.float32, name="o_t")
            nc.vector.tensor_scalar(out=o_t[:], in0=u[:], scalar1=recip[:, 0:1], scalar2=None,
                                    op0=mybir.AluOpType.mult)
            nc.vector.tensor_tensor(out=o_t[:], in0=o_t[:], in1=b2_sb[:], op=mybir.AluOpType.add)
            mx = pool.tile([P, 1], mybir.dt.float32, name="mx")
            nc.vector.tensor_reduce(out=mx[:], in_=o_t[:], axis=mybir.AxisListType.X,
                                    op=mybir.AluOpType.max)
            nmx = pool.tile([P, 1], mybir.dt.float32, name="nmx")
            nc.vector.tensor_scalar(out=nmx[:], in0=mx[:], scalar1=-1.0, scalar2=None,
                                    op0=mybir.AluOpType.mult)
            eo = pool.tile([P, fout], mybir.dt.float32, name="eo")
            se = pool.tile([P, 1], mybir.dt.float32, name="se")
            nc.scalar.activation(eo[:], o_t[:], mybir.ActivationFunctionType.Exp,
                                 bias=nmx[:, 0:1], accum_out=se[:])
            ls = pool.tile([P, 1], mybir.dt.float32, name="ls")
            nc.scalar.activation(ls[:], se[:], mybir.ActivationFunctionType.Ln)
            shift = pool.tile([P, 1], mybir.dt.float32, name="shift")
            nc.vector.tensor_tensor(out=shift[:], in0=mx[:], in1=ls[:], op=mybir.AluOpType.add)
            res = pool.tile([P, fout], mybir.dt.float32, name="res")
            nc.vector.tensor_scalar(out=res[:], in0=o_t[:], scalar1=shift[:, 0:1], scalar2=None,
                                    op0=mybir.AluOpType.subtract)
            nc.sync.dma_start(out_o[g * P:(g + 1) * P, :], res[:])

        _edge_phase(nc, pool, sched, dict(
            fd=fout, heads=1, dh=fout,
            t_lo=hl_tab[0:min(SPLIT, n_tab_pad), :],
            t_hi=hl_tab[SPLIT:n_tab_pad, :] if n_tab_pad > SPLIT else hl_tab[0:P, :],
            xr_tab=hr_tab,
            idx_lo_sb=idx_lo_sb, idx_hi_sb=idx_hi_sb, mask_sb=mask_sb,
            att_sb=att_sb, emit=emit))
    nc.compile()
    return nc


# --------------------------------------------------------------------------
# top-level
# --------------------------------------------------------------------------

def kernel(x, edge_index, Wl1, Wr1, att1, b1, Wl2, Wr2, att2, b2):
    n, din = x.shape
    f1 = Wl1.shape[1]          # 128
    f2 = Wl2.shape[1]          # 64
    heads, dh = att1.shape     # 8, 16

    sched = _prep_graph(np.asarray(edge_index, dtype=np.int64), n)
    node_list = sched["node_list"]
    n_core = sched["n_pad_nodes"]

    x_np = np.asarray(x, dtype=np.float32)
    nid_safe = [np.maximum(node_list[c], 0) for c in range(NCORES)]
    valid = [(node_list[c] >= 0) for c in range(NCORES)]

    # ---------------- launch A: x -> XL, XR (per-core nodes) ----------------
    ncA = _build_transform1(n_core, din, f1)
    wl1b = np.ascontiguousarray(Wl1.astype(BF16))
    wr1b = np.ascontiguousarray(Wr1.astype(BF16))
    in_maps = []
    for c in range(NCORES):
        xc = x_np[nid_safe[c]] * valid[c][:, None]
        in_maps.append({
            "xT": np.ascontiguousarray(xc.T.astype(BF16)),
            "wl": wl1b, "wr": wr1b,
        })
    resA = _run(ncA, in_maps)

    xl_tab = np.zeros((n, f1), dtype=BF16)
    xr_core = []
    for c in range(NCORES):
        xl_tab[node_list[c][valid[c]]] = resA[c]["xl"][valid[c]]
        xr_core.append(resA[c]["xr"])

    # ---------------- launch B: layer-1 edges -> h --------------------------
    ncB = _build_layer1(sched, n)
    att_tile = np.tile(att1.reshape(1, heads * dh).astype(BF16), (P, 1))
    b1_tile = np.tile(b1.reshape(1, f1).astype(np.float32), (P, 1))
    t_lo = np.ascontiguousarray(xl_tab[:min(SPLIT, n)])
    hi_rows = max(n - SPLIT, P)
    t_hi = np.zeros((hi_rows, f1), dtype=BF16)
    if n > SPLIT:
        t_hi[:n - SPLIT] = xl_tab[SPLIT:]
    in_maps = []
    for c in range(NCORES):
        in_maps.append({
            "t_lo": t_lo, "t_hi": t_hi,
            "xrc": xr_core[c],
            "idx_lo": _pad_cols(_concat_idx(sched["idx_lo"][c]), 16),
            "idx_hi": _pad_cols(_concat_idx(sched["idx_hi"][c]), 16),
            "mask": np.ascontiguousarray(
                np.concatenate(sched["masks"][c], axis=1).astype(BF16)),
            "att": att_tile, "b1": b1_tile,
        })
    resB = _run(ncB, in_maps)

    h_full = np.zeros((n, f1), dtype=np.float32)
    for c in range(NCORES):
        h_full[node_list[c][valid[c]]] = resB[c]["h"][valid[c]]
    n_tab_pad = -(-n // (4 * P)) * (4 * P)
    hT_full = np.zeros((f1, n_tab_pad), dtype=BF16)
    hT_full[:, :n] = h_full.T.astype(BF16)

    # ---------------- launch C: layer-2 + log_softmax -----------------------
    ncC = _build_layer2(sched, n, f1, f2)
    wl2p = np.zeros((f1, P), dtype=BF16)
    wl2p[:, :f2] = Wl2.astype(BF16)
    wr2p = np.zeros((f1, P), dtype=BF16)
    wr2p[:, :f2] = Wr2.astype(BF16)
    att2_tile = np.tile(att2.reshape(1, f2).astype(BF16), (P, 1))
    b2_tile = np.tile(b2.reshape(1, f2).astype(np.float32), (P, 1))
    in_maps = []
    for c in range(NCORES):
        hc = h_full[nid_safe[c]] * valid[c][:, None]
        in_maps.append({
            "hTf": hT_full,
            "hTc": np.ascontiguousarray(hc.T.astype(BF16)),
            "wl": wl2p, "wr": wr2p,
            "idx_lo": _pad_cols(_concat_idx(sched["idx_lo"][c]), 16),
            "idx_hi": _pad_cols(_concat_idx(sched["idx_hi"][c]), 16),
            "mask": np.ascontiguousarray(
                np.concatenate(sched["masks"][c], axis=1).astype(BF16)),
            "att": att2_tile, "b2": b2_tile,
        })
    resC = _run(ncC, in_maps)

    out = np.zeros((n, f2), dtype=np.float32)
    for c in range(NCORES):
        out[node_list[c][valid[c]]] = resC[c]["out"][valid[c]]
    return out
